# revision 2
# baseline (speedup 1.0000x reference)
"""Trainium2 Bass kernel for AspectNeighborAttention (gnn_message_passing).

Pure data-parallel over batch: 32 batches -> 8 NeuronCores x 4 batches.
All weights replicated, host-converted to bf16 and host-PRE-TRANSPOSED into
the chunk-major [128, KC, *] lhsT/rhs layouts the TensorEngine wants, so the
device does plain contiguous DMAs only. dep is host-bf16 (halves HBM traffic).

Per-core dataflow for each batch b (L=128 tokens, H=768, E=64, KC=6):
  zs^T   = Wz @ bertS^T + bz            (PE, bf16, packed PSUM groups)
  s_i,s_j= [wa_i;wa_j] @ zs^T           (PE, packed [1,128] regions)
  s_e    = reduce_e(dep * wa_e)         (DVE bf16 2x passes)
  score  = lrelu(s_i + s_j + s_e + ba)  (PE rank-1 bcast + DVE + ACT)
  attn   = mask * softmax(...)          (additive-shift masking, exp on ACT)
  D      = reduce_j(attn * dep)         (mult split Pool/DVE, bf16 2x reduce)
  nbr^T  = per-h-chunk matmuls from A/attn^T/D^T   (PE)
  temp   = nbr @ WhN^T + zs @ WhZ^T     (PE)
  out    = upd ? temp : bert            (blend, row-rolled DMA store)

The roll(z,-1)/roll(out,+1) pair is handled purely with shifted-row DMAs.
"""

import sys

for _p in ("/opt/trn_rl_repo",):
    if _p not in sys.path:
        sys.path.insert(0, _p)

import os
import numpy as np
import ml_dtypes

import concourse.bass as bass
import concourse.bacc as bacc_mod
import concourse.mybir as mybir
import concourse.tile as tile
from concourse.masks import make_identity

B, L, H, E = 32, 128, 768, 64
NCORES = 8
PB = B // NCORES  # batches per core
KC = H // 128     # 6 k-chunks
F32 = mybir.dt.float32
BF16 = mybir.dt.bfloat16
AF = mybir.ActivationFunctionType
OP = mybir.AluOpType
AX = mybir.AxisListType
MASK_SHIFT = 10000.0  # additive mask offset (see score masking)

_CACHED = {}

CFG = dict(
    dep_bufs=int(os.environ.get("K_DEP_BUFS", 3)),
    ttmp_bufs=int(os.environ.get("K_TTMP_BUFS", 3)),
    spool_bufs=int(os.environ.get("K_SPOOL_BUFS", 3)),
    opool_bufs=int(os.environ.get("K_OPOOL_BUFS", 2)),
    ptr_bufs=int(os.environ.get("K_PTR_BUFS", 2)),
    pbig_bufs=int(os.environ.get("K_PBIG_BUFS", 2)),
    jp=int(os.environ.get("K_JP", 64)),  # D-mult j-split: [0,jp) Pool, rest DVE
)


def _build(debug=False):
    nc = bacc_mod.Bacc("TRN2", target_bir_lowering=False, debug=False,
                       num_devices=NCORES)

    bert = nc.dram_tensor("bert", [PB, L, H], F32, kind="ExternalInput")
    bertsT = nc.dram_tensor("bertsT", [PB, 128, KC, 128], BF16,
                            kind="ExternalInput")
    dep = nc.dram_tensor("dep", [PB, L, L, E], BF16, kind="ExternalInput")
    adjf = nc.dram_tensor("adjf", [PB, L, L], F32, kind="ExternalInput")
    vrow = nc.dram_tensor("vrow", [1, PB, 128], F32, kind="ExternalInput")
    wzT_d = nc.dram_tensor("wzT", [128, KC, H], BF16, kind="ExternalInput")
    wfzT_d = nc.dram_tensor("wfzT", [128, KC, H], BF16, kind="ExternalInput")
    whnT_d = nc.dram_tensor("whnT", [128, KC, H], BF16, kind="ExternalInput")
    whzT_d = nc.dram_tensor("whzT", [128, KC, H], BF16, kind="ExternalInput")
    wfeT_d = nc.dram_tensor("wfeT", [E, H], BF16, kind="ExternalInput")
    w2T_d = nc.dram_tensor("w2T", [128, KC, 2], BF16, kind="ExternalInput")
    bzt = nc.dram_tensor("bzt", [1, H], BF16, kind="ExternalInput")
    wae = nc.dram_tensor("wae", [1, E], BF16, kind="ExternalInput")
    bat = nc.dram_tensor("bat", [1, 1], F32, kind="ExternalInput")
    out = nc.dram_tensor("out", [PB, L, H], F32, kind="ExternalOutput")

    dbg = {}
    if debug:
        for nm, shape, dt in [
            ("d_zsT", [128, KC, 128], BF16), ("d_si", [1, 128], F32),
            ("d_sjb", [1, 128], F32), ("d_se", [128, L], BF16),
            ("d_masked", [128, L], F32), ("d_attn", [128, L], BF16),
            ("d_dvec", [128, E], BF16), ("d_ab", [128, H], BF16),
            ("d_nbrT", [128, KC, 128], BF16), ("d_tempb", [128, H], F32),
            ("d_upd", [128, 1], F32), ("d_scb", [128, 128], F32),
        ]:
            dbg[nm] = nc.dram_tensor(nm, shape, dt, kind="ExternalOutput")
    with tile.TileContext(nc) as tc:
        with nc.allow_low_precision("bf16 softmax/D path, 2e-2 rel-err gate"):
            _body(tc, nc, bert, bertsT, dep, adjf, vrow, wzT_d, wfzT_d,
                  whnT_d, whzT_d, wfeT_d, w2T_d, bzt, wae, bat, out, dbg)
    nc.compile()
    return nc


def _body(tc, nc, bert, bertsT, dep, adjf, vrow, wzT_d, wfzT_d,
          whnT_d, whzT_d, wfeT_d, w2T_d, bzt, wae, bat, out, dbg=None):
    def dump(name, ap):
        if dbg and name in dbg:
            nc.sync.dma_start(dbg[name][...], ap)
    import contextlib
    cfg = CFG
    JP = cfg["jp"]
    ctx = contextlib.ExitStack()
    with ctx:
        wpool = ctx.enter_context(tc.tile_pool(name="weights", bufs=1))
        dpool = ctx.enter_context(
            tc.tile_pool(name="dep", bufs=cfg["dep_bufs"]))
        tpool = ctx.enter_context(
            tc.tile_pool(name="ttmp", bufs=cfg["ttmp_bufs"]))
        spool = ctx.enter_context(
            tc.tile_pool(name="small", bufs=cfg["spool_bufs"]))
        opool = ctx.enter_context(
            tc.tile_pool(name="outp", bufs=cfg["opool_bufs"]))
        p_tr = ctx.enter_context(
            tc.tile_pool(name="p_tr", bufs=cfg["ptr_bufs"], space="PSUM"))
        p_big = ctx.enter_context(
            tc.tile_pool(name="p_big", bufs=cfg["pbig_bufs"], space="PSUM"))

        # ---------------- one-time setup (plain DMAs only) ----------------
        wzT = wpool.tile([128, KC, H], BF16, tag="wzT")
        nc.sync.dma_start(wzT[:], wzT_d[...])
        wfzT = wpool.tile([128, KC, H], BF16, tag="wfzT")
        nc.sync.dma_start(wfzT[:], wfzT_d[...])
        whnT = wpool.tile([128, KC, H], BF16, tag="whnT")
        nc.sync.dma_start(whnT[:], whnT_d[...])
        whzT = wpool.tile([128, KC, H], BF16, tag="whzT")
        nc.sync.dma_start(whzT[:], whzT_d[...])
        wfeT = wpool.tile([E, H], BF16, tag="wfeT")
        nc.sync.dma_start(wfeT[:], wfeT_d[...])
        w2T = wpool.tile([128, KC, 2], BF16, tag="w2T")
        nc.sync.dma_start(w2T[:], w2T_d[...])
        bzr = wpool.tile([1, H], BF16, tag="bzr")
        nc.sync.dma_start(bzr[:], bzt[:, :])
        waer = wpool.tile([1, E], BF16, tag="waer")
        nc.sync.dma_start(waer[:], wae[:, :])
        bar = wpool.tile([1, 1], F32, tag="bar")
        nc.sync.dma_start(bar[:], bat[:, :])
        vrow4 = wpool.tile([1, PB, 128], F32, tag="vrow4")
        nc.sync.dma_start(vrow4[:], vrow[:, :, :])

        ones_f = wpool.tile([1, 128], F32, tag="ones_f")
        nc.gpsimd.memset(ones_f[:], 1.0)
        ones_b = wpool.tile([1, 128], BF16, tag="ones_b")
        nc.gpsimd.memset(ones_b[:], 1.0)
        id_bf = wpool.tile([128, 128], BF16, tag="id_bf")
        make_identity(nc, id_bf[:])

        # wa_e broadcast to all partitions via rank-1 matmul
        p_wae = p_tr.tile([128, 384], F32, tag="p_tr")
        nc.tensor.matmul(p_wae[:, 0:E], ones_b[:], waer[:],
                         start=True, stop=True)
        wae_bc = wpool.tile([128, E], BF16, tag="wae_bc")
        nc.scalar.copy(wae_bc[:], p_wae[:, 0:E])

        # ---------------- per-batch pipeline ----------------
        for b in range(PB):
            # bertS: rows shifted by one token (z roll); f32 exact for blend
            bertS = spool.tile([128, H], F32, tag="bertS")
            nc.sync.dma_start(bertS[0:127, :], bert[b, 1:128, :])
            nc.sync.dma_start(bertS[127:128, :], bert[b, 0:1, :])
            bertST = spool.tile([128, KC, 128], BF16, tag="bertST")
            nc.sync.dma_start(bertST[:], bertsT[b, :, :, :])

            dept = dpool.tile([128, L, E], BF16, tag="dept")
            nc.sync.dma_start(dept[:], dep[b, :, :, :])
            adjt = spool.tile([128, L], F32, tag="adjt")
            nc.sync.dma_start(adjt[:], adjf[b, :, :])

            # ---- zs^T = Wz @ bertS^T + bz: 6 groups packed in one PSUM ----
            p_z = p_big.tile([128, H], F32, tag="p_big")
            for hc in range(KC):
                ns = slice(hc * 128, (hc + 1) * 128)
                for kc in range(KC):
                    nc.tensor.matmul(p_z[:, ns], wzT[:, kc, ns],
                                     bertST[:, kc, :],
                                     start=(kc == 0), stop=False)
                nc.tensor.matmul(p_z[:, ns], bzr[0:1, ns], ones_b[:],
                                 start=False, stop=True)
            zsT = spool.tile([128, KC, 128], BF16, tag="zsT")
            nc.scalar.copy(zsT[:], p_z[:])
            if b == 0:
                dump("d_zsT", zsT[:])

            # ---- s_i, s_j, score-base packed into one p_tr tile ----
            p_s3 = p_tr.tile([128, 384], F32, tag="p_tr")
            for kc in range(KC):
                nc.tensor.matmul(p_s3[0:1, 0:128], w2T[:, kc, 0:1],
                                 zsT[:, kc, :],
                                 start=(kc == 0), stop=(kc == KC - 1))
            for kc in range(KC):
                nc.tensor.matmul(p_s3[0:1, 128:256], w2T[:, kc, 1:2],
                                 zsT[:, kc, :],
                                 start=(kc == 0), stop=(kc == KC - 1))
            si_row = spool.tile([1, 128], F32, tag="si_row")
            nc.scalar.copy(si_row[:], p_s3[0:1, 0:128])
            sjb = spool.tile([1, 128], F32, tag="sjb")
            nc.vector.tensor_scalar(sjb[:], p_s3[0:1, 128:256], bar[0:1, 0:1],
                                    None, op0=OP.add)
            # score base: s_i (row-bcast) + (s_j + ba) (col-bcast)
            nc.tensor.matmul(p_s3[:, 256:384], si_row[:], ones_f[:],
                             start=True, stop=False)
            nc.tensor.matmul(p_s3[:, 256:384], ones_f[:], sjb[:],
                             start=False, stop=True)
            if b == 0:
                dump("d_si", si_row[:])
                dump("d_sjb", sjb[:])

            # ---- s_e = reduce_e(dep * wa_e), bf16 2x passes ----
            tmp1 = tpool.tile([128, L, E], BF16, tag="ttmp")
            nc.vector.tensor_tensor(
                tmp1[:], dept[:],
                wae_bc[:].unsqueeze(1).broadcast_to([128, L, E]), op=OP.mult)
            se = spool.tile([128, L], BF16, tag="se")
            nc.vector.tensor_reduce(se[:], tmp1[:], axis=AX.X, op=OP.add)
            if b == 0:
                dump("d_se", se[:])
                scb_s = spool.tile([128, 128], F32, tag="scb_s")
                nc.vector.tensor_copy(scb_s[:], p_s3[:, 256:384])
                dump("d_scb", scb_s[:])

            # ---- score = lrelu(se + base); masked = (score+C)*m ----
            sadd = spool.tile([128, L], F32, tag="sadd")
            nc.vector.tensor_tensor(sadd[:], se[:], p_s3[:, 256:384],
                                    op=OP.add)
            score = spool.tile([128, L], F32, tag="score")
            nc.scalar.activation(score[:], sadd[:], AF.Lrelu, alpha=0.01)
            masked = spool.tile([128, L], F32, tag="masked")
            nc.vector.scalar_tensor_tensor(
                masked[:], score[:], MASK_SHIFT, adjt[:],
                op0=OP.add, op1=OP.mult)
            if b == 0:
                dump("d_masked", masked[:])

            # ---- softmax over j (free axis); attn emitted directly bf16 ----
            mxn = spool.tile([128, 1], F32, tag="mxn")
            nc.vector.tensor_reduce(mxn[:], masked[:], axis=AX.X, op=OP.max,
                                    negate=True)
            ex = spool.tile([128, L], F32, tag="ex")
            sumex = spool.tile([128, 1], F32, tag="sumex")
            nc.scalar.activation(ex[:], masked[:], AF.Exp, bias=mxn[:],
                                 scale=1.0, accum_out=sumex[:])
            rec = spool.tile([128, 1], F32, tag="rec")
            nc.vector.reciprocal(rec[:], sumex[:])
            attnb = spool.tile([128, L], BF16, tag="attnb")
            nc.vector.scalar_tensor_tensor(
                attnb[:], ex[:], rec[:], adjt[:], op0=OP.mult, op1=OP.mult)
            if b == 0:
                dump("d_attn", attnb[:])

            # ---- D = reduce_j(attn * dep); mult split Pool/DVE ----
            tmp2 = tpool.tile([128, E, L], BF16, tag="ttmp")
            if JP > 0:
                nc.gpsimd.tensor_tensor(
                    tmp2[:, :, 0:JP],
                    dept[:, 0:JP, :].rearrange("p j e -> p e j"),
                    attnb[:, 0:JP].unsqueeze(1).broadcast_to([128, E, JP]),
                    op=OP.mult)
            if JP < L:
                nc.vector.tensor_tensor(
                    tmp2[:, :, JP:L],
                    dept[:, JP:L, :].rearrange("p j e -> p e j"),
                    attnb[:, JP:L].unsqueeze(1).broadcast_to(
                        [128, E, L - JP]), op=OP.mult)
            dvb = spool.tile([128, E], BF16, tag="dvb")
            nc.vector.tensor_reduce(dvb[:], tmp2[:], axis=AX.X, op=OP.add)
            if b == 0:
                dump("d_dvec", dvb[:])

            # attn^T and D^T via PE transposes into one packed PSUM tile
            p_ad = p_tr.tile([128, 256], BF16, tag="p_tr")
            nc.tensor.transpose(p_ad[:, 0:128], attnb[:], id_bf[:])
            nc.tensor.transpose(p_ad[0:E, 128:256], dvb[:], id_bf[:])
            attnT = spool.tile([128, 128], BF16, tag="attnT")
            nc.scalar.copy(attnT[:], p_ad[:, 0:128])
            dT = spool.tile([E, 128], BF16, tag="dT")
            nc.scalar.copy(dT[:], p_ad[0:E, 128:256])

            # ---- A = zs @ WfZ^T  ([j, h], bf16) ----
            p_a = p_big.tile([128, H], F32, tag="p_big")
            for ns in (slice(0, 512), slice(512, H)):
                for kc in range(KC):
                    nc.tensor.matmul(p_a[:, ns], zsT[:, kc, :],
                                     wfzT[:, kc, ns],
                                     start=(kc == 0), stop=(kc == KC - 1))
            ab = spool.tile([128, H], BF16, tag="ab")
            nc.scalar.copy(ab[:], p_a[:])
            if b == 0:
                dump("d_ab", ab[:])

            # ---- nbr^T per h-chunk packed into one PSUM tile ----
            p_n = p_big.tile([128, H], F32, tag="p_big")
            for hc in range(KC):
                ns = slice(hc * 128, (hc + 1) * 128)
                nc.tensor.matmul(p_n[:, ns], ab[:, ns], attnT[:],
                                 start=True, stop=False)
                nc.tensor.matmul(p_n[:, ns], wfeT[:, ns], dT[:],
                                 start=False, stop=True)
            nbrT = spool.tile([128, KC, 128], BF16, tag="nbrT")
            nc.scalar.copy(nbrT[:], p_n[:])
            if b == 0:
                dump("d_nbrT", nbrT[:])

            # ---- temp = nbr @ WhN^T + zs @ WhZ^T ----
            p_t = p_big.tile([128, H], F32, tag="p_big")
            for ns in (slice(0, 512), slice(512, H)):
                for kc in range(KC):
                    nc.tensor.matmul(p_t[:, ns], nbrT[:, kc, :],
                                     whnT[:, kc, ns],
                                     start=(kc == 0), stop=False)
                for kc in range(KC):
                    nc.tensor.matmul(p_t[:, ns], zsT[:, kc, :],
                                     whzT[:, kc, ns],
                                     start=False, stop=(kc == KC - 1))
            tempb = opool.tile([128, H], F32, tag="tempb")
            nc.scalar.copy(tempb[:], p_t[:])
            if b == 0:
                dump("d_tempb", tempb[:])

            # ---- upd mask: span-row (host) -> column via rank-1; & any_j ----
            p_v = p_tr.tile([128, 384], F32, tag="p_tr")
            nc.tensor.matmul(p_v[:, 0:1], vrow4[0:1, b, :], ones_f[0:1, 0:1],
                             start=True, stop=True)
            anynb = spool.tile([128, 1], F32, tag="anynb")
            nc.vector.tensor_reduce(anynb[:], adjt[:], axis=AX.X, op=OP.max)
            upd = spool.tile([128, 1], F32, tag="upd")
            nc.vector.tensor_tensor(upd[:], p_v[:, 0:1], anynb[:], op=OP.mult)
            if b == 0:
                dump("d_upd", upd[:])

            # ---- blend + rolled store ----
            tdiff = opool.tile([128, H], F32, tag="tdiff")
            nc.gpsimd.tensor_tensor(tdiff[:], tempb[:], bertS[:],
                                    op=OP.subtract)
            outt = opool.tile([128, H], F32, tag="outt")
            nc.vector.scalar_tensor_tensor(
                outt[:], tdiff[:], upd[:], bertS[:], op0=OP.mult, op1=OP.add)
            nc.sync.dma_start(out[b, 1:128, :], outt[0:127, :])
            nc.sync.dma_start(out[b, 0:1, :], outt[127:128, :])


def _get_nc():
    if "nc" not in _CACHED:
        _CACHED["nc"] = _build()
    return _CACHED["nc"]


def _chunkT(w):
    """W [rows, K] -> W^T chunk-major [128, K//128, rows] (lhsT layout)."""
    rows, k = w.shape
    return np.ascontiguousarray(
        w.T.reshape(k // 128, 128, rows).transpose(1, 0, 2))


def _prep_in_maps(bert_hidden_states, dep_type_adj, deprel_adj,
                  asp_start, asp_end, Wz, bz, wa, ba, Wf, Wh):
    bf = ml_dtypes.bfloat16
    bert = np.ascontiguousarray(np.asarray(bert_hidden_states, np.float32))
    dep = np.asarray(dep_type_adj, np.float32).astype(bf)
    adjf = np.ascontiguousarray(np.asarray(deprel_adj).astype(np.float32))
    # bertS^T chunk-major per batch: rows shifted by one (the z-roll)
    bs = np.roll(bert, -1, axis=1)
    bertsT = np.ascontiguousarray(
        bs.transpose(0, 2, 1).reshape(B, KC, 128, L).transpose(0, 2, 1, 3)
    ).astype(bf)
    pos = np.arange(L, dtype=np.float32)
    s_ = np.asarray(asp_start).astype(np.float32)[:, None]
    e_ = np.asarray(asp_end).astype(np.float32)[:, None]
    vrow_full = ((pos[None, :] >= s_) & (pos[None, :] <= e_)).astype(np.float32)

    Wz = np.asarray(Wz, np.float32)
    Wf = np.asarray(Wf, np.float32)
    Wh = np.asarray(Wh, np.float32)
    wa_f = np.asarray(wa, np.float32)
    wzT = _chunkT(Wz).astype(bf)
    wfzT = _chunkT(Wf[:, :H]).astype(bf)
    whnT = _chunkT(Wh[:, :H]).astype(bf)
    whzT = _chunkT(Wh[:, H:]).astype(bf)
    wfeT = np.ascontiguousarray(Wf[:, H:].T).astype(bf)
    w2T = _chunkT(wa_f[:2 * H].reshape(2, H)).astype(bf)
    bzb = np.asarray(bz, np.float32)[None, :].astype(bf)
    waeb = wa_f[2 * H:][None, :].astype(bf)
    bab = np.asarray(ba, np.float32).reshape(1, 1)

    in_maps = []
    for c in range(NCORES):
        s = slice(c * PB, (c + 1) * PB)
        in_maps.append(dict(
            bert=bert[s], bertsT=np.ascontiguousarray(bertsT[s]),
            dep=dep[s], adjf=adjf[s],
            vrow=np.ascontiguousarray(vrow_full[s][None, :, :]),
            wzT=wzT, wfzT=wfzT, whnT=whnT, whzT=whzT, wfeT=wfeT, w2T=w2T,
            bzt=bzb, wae=waeb, bat=bab,
        ))
    return in_maps


def kernel(bert_hidden_states, dep_type_adj, deprel_adj, asp_start, asp_end,
           Wz, bz, wa, ba, Wf, Wh):
    from concourse.bass_utils import run_bass_kernel_spmd

    in_maps = _prep_in_maps(bert_hidden_states, dep_type_adj, deprel_adj,
                            asp_start, asp_end, Wz, bz, wa, ba, Wf, Wh)
    nc = _get_nc()
    res = run_bass_kernel_spmd(nc, in_maps, core_ids=list(range(NCORES)),
                               trace=bool(_CACHED.get("trace")),
                               tmpdir=_CACHED.get("trace_tmpdir"))
    _CACHED["last_results"] = res
    outs = [res.results[c]["out"] for c in range(NCORES)]
    return np.concatenate(outs, axis=0).astype(np.float32)

